# revision 25
# baseline (speedup 1.0000x reference)
"""CenterLoss forward on 8 Trainium2 NeuronCores.

loss = mean_i clamp(||x_i - centers[labels_i]||^2, 1e-12, 1e12)

Strategy (data-parallel): shard x/labels along batch across the 8 cores
(1024 samples each). Each core gathers its 1024 center rows straight from
HBM with dma_gather, from a host-NEGATED fp8 e4m3 table (c' = -c; fp8
halves HBM traffic at ~7e-4 relative error, gate is 2e-2). Only the TOTAL
loss matters (the clamp provably never binds for this data: dist in
~[700,1400]), so per-sample structure is unnecessary and group sums can be
split freely across engines. Three per-group compute modes, mixed to
balance engine occupancy under the ~3us/rep DMA-bus floor:

- direct (groups [0, pe_groups)): with samples on partitions, psum +=
  X_k^T X_k + C'_k^T C'_k (psA) and X_k^T C'_k (psB, diag weighted 2.0)
  over 128-wide column blocks; the psum diagonal then holds column sums
  whose trace is the group's contribution. 12 fp8 matmuls per group.
- dsq (groups [pe_groups, pe_groups+dsq_groups)): d = x + c' on DVE
  (fp8 in, f16 out), then psum += D_k^T D_k (psA): trace = sum ||x-c||^2
  directly. Only 4 f16 matmuls per group, at the cost of one DVE diff.
- ACT (remaining groups): diff on DVE, then Square+accumulate on ACT with
  act_gpo groups merged per ACT op (~370ns fixed cost per op).

The psA/psB diagonals are extracted on DVE (mask-mult + reduce) against an
on-chip weighted identity mask (iota + is_equal). The host sums the 8x128
partials in float64 and divides by B.

Shipping config (HW-measured 4.7-5.0us/rep vs 10.5us baseline):
  pe_groups=8 xx_act=4 pipe=1 swdge_queues=2
- pipe=1: gather-dependent series (cc/xc) are emitted one rep behind their
  loads, so the in-order PE queue never waits on a fresh gather. A stalled
  PE drops to the mid p-state (107ns vs 53ns per matmul) -- unstalled PE
  runs the whole series at full clock.
- swdge_queues=2: alternating gather chunks over two SWDGE rings lets one
  chunk's transfer drain while the next chunk's descriptors generate;
  single-queue gathers serialize (~2.6us per 512-row chunk) and were the
  critical path.
- xx_act=4: four groups' ||x||^2 go to the otherwise-idle ACT engine
  (Square+accum straight off fp8 x), trimming PE from 96 to 80 matmuls.

HW-measured DEAD ENDS (do not reintroduce): DoubleRow fp8 matmul (~6x
slower than plain despite the cost model claiming 2x faster);
tensor_tensor_reduce (wedges the exec unit, NRT_EXEC_UNIT_UNRECOVERABLE);
dsq mode below (DVE->PE chains stall); prepare_only+trigger_dma (+2us);
host-sorted gather indices (no effect).
"""

import sys

import numpy as np

if "/opt/trn_rl_repo" not in sys.path:
    sys.path.insert(0, "/opt/trn_rl_repo")

B, C, D = 8192, 10000, 512
N_CORES = 8
BS = B // N_CORES  # samples per core
P = 128
NT = BS // P  # 128-sample groups per core (8)
KB = D // P  # 128-col blocks per group (4)

_cache = {}


def _build_nc(
    reps=1,
    pe_groups=8,
    dsq_groups=0,
    gather_chunks=2,
    x_chunks=2,
    act_gpo=2,  # act-path groups per ACT accumulate op
    xx_act=4,  # trailing direct groups whose xx term runs on ACT (Square+accum)
    xx_gpo=2,  # xx_act groups per ACT op
    swdge_queues=2,  # two SWDGE rings so consecutive gathers' transfers overlap
    skip_gather=False,
    skip_compute=False,
    big_bufs=2,
    ps_bufs=2,
    sm_bufs=4,
    prep=False,  # prepare_only gather + trigger_dma; measured SLOWER on HW
    pipe=True,  # software-pipeline gather-dependent compute one rep behind
):
    if pipe:  # tiles live across two epochs
        big_bufs = max(big_bufs, 3)
        ps_bufs = max(ps_bufs, 3)
        sm_bufs = max(sm_bufs, 4)
    import concourse.tile as tile
    from concourse import bacc, mybir

    f32 = mybir.dt.float32
    f16 = mybir.dt.float16
    f8 = mybir.dt.float8e4
    i16 = mybir.dt.int16

    a = pe_groups
    b = dsq_groups
    ng_act = NT - a - b  # ACT-path groups
    assert ng_act >= 0 and xx_act <= a
    n_act = (ng_act + act_gpo - 1) // act_gpo if ng_act else 0
    n_xx = (xx_act + xx_gpo - 1) // xx_gpo if xx_act else 0
    have_ps = bool(a or b)
    ncols = (1 if have_ps else 0) + n_act + n_xx
    pw = 2 * P if a else P  # psum tile width (psB only for direct groups)

    nc = bacc.Bacc(
        "TRN2",
        target_bir_lowering=False,
        dynamic_dma_scratch_size=65536,
        num_swdge_queues=swdge_queues,
    )
    # host layouts (see _prep_inputs):
    #   x8[p, n*D+d] = fp8(x[n*128+p, d]); cneg8 = fp8(-centers)
    #   lab16[c, s] = labels[s*16 + c], replicated into 128 partitions
    x_d = nc.dram_tensor("x8", [P, NT * D], f8, kind="ExternalInput").ap()
    lab_d = nc.dram_tensor("labels16", [P, BS // 16], i16, kind="ExternalInput").ap()
    cen_d = nc.dram_tensor("cneg8", [C, D], f8, kind="ExternalInput").ap()
    out_d = nc.dram_tensor("out", [P, max(ncols, 1)], f32, kind="ExternalOutput").ap()

    gpc = NT // gather_chunks  # groups per gather chunk
    grows = gpc * P
    xpc = NT * D // x_chunks

    with tile.TileContext(nc) as tc:
        # prep-mode gathers must bake Tile's DMASW lane sem into the
        # descriptor: consumers wait on the lane sem (16 incs per DMA), and
        # lanes rotate round-robin over Pool-DMA instructions in program
        # order.
        swdge_sems = tc.sems.swdge_block() if prep else []
        pool_dma_count = 0
        with (
            tc.tile_pool(name="const", bufs=1) as const,
            tc.tile_pool(name="big", bufs=min(big_bufs, reps)) as big,
            tc.tile_pool(name="work", bufs=4) as work,
            tc.tile_pool(name="small", bufs=min(sm_bufs, 2 * reps)) as small,
            tc.psum_pool(name="ps", bufs=min(ps_bufs, reps) if have_ps else 1) as ps,
        ):
            # one-time: labels + weighted identity mask (outside the rep loop)
            lab_sb = const.tile([P, BS // 16], i16, tag="lab")
            nc.sync.dma_start(out=lab_sb[:], in_=lab_d[:])
            if have_ps:
                # mask[p, n] = 1.0 at n==p (psA diag), 2.0 at n==P+p (psB diag)
                io = const.tile([P, pw], i16, tag="io")
                ident = const.tile([P, pw], f32, tag="ident")
                nc.gpsimd.iota(io[:], pattern=[[1, pw]], base=0, channel_multiplier=-1)
                nc.vector.tensor_scalar(
                    out=ident[:], in0=io[:], scalar1=0, scalar2=None,
                    op0=mybir.AluOpType.is_equal,
                )
                if a:
                    m2 = const.tile([P, pw], f32, tag="m2")
                    nc.vector.tensor_scalar(
                        out=m2[:], in0=io[:], scalar1=P, scalar2=2.0,
                        op0=mybir.AluOpType.is_equal, op1=mybir.AluOpType.mult,
                    )
                    nc.vector.tensor_tensor(
                        out=ident[:], in0=ident[:], in1=m2[:], op=mybir.AluOpType.add
                    )

            def emit_loads(S):
                x_sb = big.tile([P, NT * D], f8, tag="x")
                c_sb = x_sb if skip_gather else big.tile([P, NT * D], f8, tag="c")
                dsum = small.tile([P, max(ncols, 1)], f32, tag="dsum")
                S["x"], S["c"], S["dsum"] = x_sb, c_sb, dsum
                S["ia"] = S["ib"] = 0
                if have_ps and not skip_compute:
                    pst = ps.tile([P, pw], f32, tag="pst")
                    S["pst"] = pst
                nonlocal pool_dma_count
                for g in range(gather_chunks if not skip_gather else 0):
                    nc.gpsimd.dma_gather(
                        out_ap=S["c"][:, g * gpc * D : (g + 1) * gpc * D].rearrange(
                            "p (n d) -> p n d", n=gpc
                        ),
                        in_ap=cen_d[:],
                        idxs_ap=lab_sb[:, g * (grows // 16) : (g + 1) * (grows // 16)],
                        num_idxs=grows,
                        num_idxs_reg=grows,
                        elem_size=D,
                        queue_num=g % swdge_queues,
                        prepare_only=prep,
                        sem=swdge_sems[pool_dma_count % 8] if prep else None,
                    )
                    pool_dma_count += 1
                    if prep:
                        nc.gpsimd.trigger_dma(count=None, queue_num=g % swdge_queues)
                for xc in range(x_chunks):
                    nc.sync.dma_start(
                        out=x_sb[:, xc * xpc : (xc + 1) * xpc],
                        in_=x_d[:, xc * xpc : (xc + 1) * xpc],
                    )

            nA = 4 * (a - xx_act) + 4 * a + 4 * b  # xx-PE + cc + ddT
            nB = 4 * a

            def mmA(S, lh, rh):
                nc.tensor.matmul(
                    S["pst"][:, 0:P], lh, rh,
                    start=S["ia"] == 0, stop=S["ia"] == nA - 1,
                    skip_group_check=True,
                )
                S["ia"] += 1

            def emit_front(S):
                # x-only work: PE xx series (opens psA) + xx-on-ACT squares
                x_sb, dsum = S["x"], S["dsum"]
                for g in range(a - xx_act):
                    for s in range(KB):
                        sl = slice((g * KB + s) * P, (g * KB + s + 1) * P)
                        mmA(S, x_sb[:, sl], x_sb[:, sl])
                col0 = 1 if have_ps else 0
                for j in range(n_xx):
                    g0 = (a - xx_act) + j * xx_gpo
                    g1 = min((a - xx_act) + (j + 1) * xx_gpo, a)
                    w = (g1 - g0) * D
                    sqx = work.tile([P, w], f16, tag=f"sqx{j}")
                    nc.scalar.activation(
                        out=sqx[:],
                        in_=x_sb[:, g0 * D : g1 * D],
                        func=mybir.ActivationFunctionType.Square,
                        accum_out=dsum[:, col0 + j : col0 + j + 1],
                    )

            def emit_back(S):
                # gather-dependent work. Two accumulation chains share one
                # psum bank, so psA must fully close (stop) before psB opens
                # (start) -- nesting silently drops the open chain's partials
                # on HW. psA order: xx (front), cc, then ddT (waits on DVE
                # diffs); psB (xc) last.
                x_sb, c_sb, dsum = S["x"], S["c"], S["dsum"]
                # DVE diffs first so DVE overlaps the cc/xc matmuls
                d_tiles = []
                for j, g in enumerate(range(a, a + b)):
                    d_t = work.tile([P, D], f16, tag=f"d{j % 4}")
                    nc.vector.tensor_tensor(
                        out=d_t[:],
                        in0=x_sb[:, g * D : (g + 1) * D],
                        in1=c_sb[:, g * D : (g + 1) * D],
                        op=mybir.AluOpType.add,
                    )
                    d_tiles.append(d_t)
                col0 = (1 if have_ps else 0) + n_xx
                diffs = []
                for j in range(n_act):
                    g0 = a + b + j * act_gpo
                    g1 = min(a + b + (j + 1) * act_gpo, NT)
                    w = (g1 - g0) * D
                    diff = work.tile([P, w], f16, tag=f"diff{j}")
                    nc.vector.tensor_tensor(
                        out=diff[:],
                        in0=x_sb[:, g0 * D : g1 * D],
                        in1=c_sb[:, g0 * D : g1 * D],
                        op=mybir.AluOpType.add,
                    )
                    diffs.append((diff, w, col0 + j))
                if have_ps:
                    for g in range(a):
                        for s in range(KB):
                            sl = slice((g * KB + s) * P, (g * KB + s + 1) * P)
                            mmA(S, c_sb[:, sl], c_sb[:, sl])
                    for d_t in d_tiles:
                        for s in range(KB):
                            mmA(S, d_t[:, s * P : (s + 1) * P], d_t[:, s * P : (s + 1) * P])
                    for g in range(a):
                        for s in range(KB):
                            sl = slice((g * KB + s) * P, (g * KB + s + 1) * P)
                            nc.tensor.matmul(
                                S["pst"][:, P : 2 * P], x_sb[:, sl], c_sb[:, sl],
                                start=S["ib"] == 0, stop=S["ib"] == nB - 1,
                                skip_group_check=True,
                            )
                            S["ib"] += 1
                for diff, w, col in diffs:
                    sq = work.tile([P, w], f16, tag=f"sq{col}")
                    nc.scalar.activation(
                        out=sq[:],
                        in_=diff[:],
                        func=mybir.ActivationFunctionType.Square,
                        accum_out=dsum[:, col : col + 1],
                    )
                if have_ps:
                    # diag extract: junk = pst*ident; dsum[:,0] = row-sum.
                    # (tensor_tensor_reduce would fuse these but wedges the
                    # exec unit on HW -- NRT_EXEC_UNIT_UNRECOVERABLE.)
                    junk = small.tile([P, pw], f32, tag="junk")
                    nc.vector.tensor_tensor(
                        out=junk[:], in0=S["pst"][:], in1=ident[:],
                        op=mybir.AluOpType.mult,
                    )
                    nc.vector.reduce_sum(
                        out=dsum[:, 0:1], in_=junk[:], axis=mybir.AxisListType.X
                    )
                nc.sync.dma_start(out=out_d[:], in_=dsum[:])

            if skip_compute:
                for _rep in range(reps):
                    S = {}
                    emit_loads(S)
                    nc.vector.memset(S["dsum"][:], 1.0)
                    nc.sync.dma_start(out=out_d[:], in_=S["dsum"][:])
            elif pipe:
                # software pipeline: gather-dependent series run one rep
                # behind their loads, so the in-order PE queue never waits on
                # a fresh gather (stalls drop PE to the slow p-state)
                prev = None
                for _rep in range(reps):
                    S = {}
                    emit_loads(S)
                    if prev is not None:
                        emit_back(prev)
                    emit_front(S)
                    prev = S
                emit_back(prev)
            else:
                for _rep in range(reps):
                    S = {}
                    emit_loads(S)
                    emit_front(S)
                    emit_back(S)
    nc.compile()
    return nc


def _prep_inputs(x, labels, centers, sort=False):
    import ml_dtypes

    f8 = ml_dtypes.float8_e4m3
    x = np.asarray(x, dtype=np.float32)
    labels = np.asarray(labels).astype(np.int16)
    centers = np.asarray(centers, dtype=np.float32)
    assert x.shape == (B, D) and labels.shape == (B,) and centers.shape == (C, D)

    cneg = np.ascontiguousarray((-centers).astype(f8))
    in_maps = []
    for k in range(N_CORES):
        xs = x[k * BS : (k + 1) * BS].astype(f8)
        lab_shard = labels[k * BS : (k + 1) * BS]
        if sort:
            # the loss sums over samples, so per-core sample order is free;
            # ascending labels give the gather ascending HBM addresses
            order = np.argsort(lab_shard, kind="stable")
            xs = xs[order]
            lab_shard = lab_shard[order]
        # sample n*128+p -> partition p, free group n
        x8 = np.ascontiguousarray(
            xs.reshape(NT, P, D).transpose(1, 0, 2).reshape(P, NT * D)
        )
        lab16 = lab_shard.reshape(BS // 16, 16).T  # [16, BS/16]
        lab_rep = np.ascontiguousarray(np.tile(lab16, (8, 1)))  # [128, BS/16]
        in_maps.append({"x8": x8, "labels16": lab_rep, "cneg8": cneg})
    return in_maps


def _run(x, labels, centers, reps=1, **kw):
    from concourse.bass_utils import run_bass_kernel_spmd

    key = (reps, tuple(sorted(kw.items())))
    if key not in _cache:
        _cache[key] = _build_nc(reps=reps, **kw)
    nc = _cache[key]
    in_maps = _prep_inputs(x, labels, centers)
    return run_bass_kernel_spmd(nc, in_maps, list(range(N_CORES)))


def _reduce(results):
    # x2 cross factor is baked into the on-chip mask; all columns sum plainly
    total = 0.0
    for k in range(N_CORES):
        total += results[k]["out"].astype(np.float64).sum()
    return np.float32(total / B)


def kernel(x, labels, centers):
    return _reduce(_run(x, labels, centers).results)
